# revision 15
# baseline (speedup 1.0000x reference)
"""Causal self-attention (QKV projection + softmax(QK^T/sqrt(N)) @ V) on 8 TRN2
NeuronCores.

Sharding: core c = 2*b + j handles batch element b (of 4) and half the query
rows: block A = rows [j*512,(j+1)*512), block B = rows [(3-j)*512,(4-j)*512)
(mirrored blocks balance the causal triangle). Uniform SPMD schedule; per-core
causal masks (built on-device from shipped position vectors) zero invalid keys.

v5 scheme (pairwise K/V dedup + partial-fp8 projections):
- Projections for the first 512 keys of each core's half and for block-A
  queries run in bf16; the rest run in fp8-e4m3 DoubleRow (host-quantized
  W*32 and context, descaled 1/32 at PSUM evict). Few-key softmax rows copy
  V/score noise straight to the output, so those stay bf16; many-key rows
  average fp8 noise away (CPU-simulated max rel err ~6e-3 vs gate 2e-2).
- Core j of each pair projects K^T/V only for its half of the keys (host ships
  that half of the context pre-transposed), then the halves are exchanged with
  a pairwise HBM AllGather; both cores read the gathered copies back in global
  key order, so all addressing stays SPMD-uniform.
- Phase order K -> V -> Q -> scores -> PV hides both collectives behind the
  Q projection and scores; PV runs block-A slots first (they only need the
  low half of V, which lands earlier).
- Scores are computed transposed S^T[k,q] = (K^T tile).T @ Q^T, softmax runs
  without max-subtraction, denominators come from a ones-vector matmul, P^T
  feeds PV directly, PV contraction is causally trimmed per q-tile slot.
"""

import math
from contextlib import ExitStack

import numpy as np

import concourse.bass as bass
import concourse.mybir as mybir
import concourse.tile as tile
from concourse.bass_utils import run_bass_kernel_spmd
from concourse.tile_rust import add_dep_helper

P = 128
CH = 512          # free-dim chunk (max fp32 moving operand / one PSUM bank)


def _fix_matmul_waits(nc):
    """Walrus codegen has a small per-instruction sync-wait slot budget (one
    for a self-loading matmul's LDWEIGHTS half, similar for ACT etc).  Move
    extra waits onto NoOps inserted just before the instruction on the same
    engine — per-engine program order (and thus semantics) is unchanged."""
    skip = (mybir.InstEventSemaphore, mybir.InstNoOp,
            mybir.InstUnconditionalBranch, mybir.InstCall)
    for func in nc.m.functions:
        for bb in func.blocks:
            il = bb.instructions
            new = []
            changed = False
            for inst in il:
                si = getattr(inst, "sync_info", None)
                if (si and si.on_wait and len(si.on_wait) > 1
                        and not isinstance(inst, skip)):
                    waits = list(si.on_wait)
                    for wi, w in enumerate(waits[:-1]):
                        nop = mybir.InstNoOp(
                            name=f"{inst.name}-wfix{wi}", engine=inst.engine,
                            sync_info=mybir.SyncInfo(on_wait=[w], on_update=[]),
                            text_hint="waitfix")
                        new.append(nop)
                    inst.sync_info = mybir.SyncInfo(
                        on_wait=[waits[-1]], on_update=list(si.on_update or []))
                    changed = True
                new.append(inst)
            if changed:
                bb.instructions = new


def build(N=2048, D=1024, fix_waits=True, **bass_kwargs):
    NT = N // P            # 16 key tiles
    NH = N // 2            # keys owned per core (1024)
    NHT = NH // P          # 8 owned key tiles
    DN = D // P            # 8 contraction / e-tiles
    QTOT = N // 2          # query rows per core (1024)
    QBLK = QTOT // 2       # rows per query block (512)
    QT = QBLK // P         # q-tiles per block (4)
    NH2 = NH // 2          # bf16-protected keys per core (512)
    DS = D // (2 * P)      # 4 fp8 DoubleRow contraction super-tiles
    SCALE = 1.0 / math.sqrt(N)
    WS = 32.0
    BF = mybir.dt.bfloat16
    F8 = mybir.dt.float8e4
    F32 = mybir.dt.float32
    AF = mybir.ActivationFunctionType
    OP = mybir.AluOpType
    DR = mybir.MatmulPerfMode.DoubleRow
    GROUPS = [[2 * b, 2 * b + 1] for b in range(4)]

    # causal PV contraction capacity per (block, q-tile) slot: max over the
    # j=0/j=1 occupant of that slot (uniform SPMD program, per-core data)
    capA = [QT + 1 + qt for qt in range(QT)]            # 5,6,7,8
    capB = [NT - 3 + qt for qt in range(QT)]            # 13,14,15,16

    nc = bass.Bass(num_devices=8, **bass_kwargs)

    ctx_bf = nc.declare_dram_parameter("ctx_bf", [DN, P, NH2], BF, isOutput=False)
    ctx_f8 = nc.declare_dram_parameter("ctx_f8", [DS, P, 2, NH2], F8, isOutput=False)
    ctxq_bf = nc.declare_dram_parameter("ctxq_bf", [DN, P, QBLK], BF, isOutput=False)
    ctxq_f8 = nc.declare_dram_parameter("ctxq_f8", [DS, P, 2, QBLK], F8, isOutput=False)
    w_bf = nc.declare_dram_parameter("w_bf", [3, DN, P, D], BF, isOutput=False)
    w_f8 = nc.declare_dram_parameter("w_f8", [3, DS, P, 2, D], F8, isOutput=False)
    qpos = nc.declare_dram_parameter("qpos", [P, QTOT], F32, isOutput=False)
    kpos = nc.declare_dram_parameter("kpos", [P, NT], F32, isOutput=False)
    onesd = nc.declare_dram_parameter("onesd", [P, 8], BF, isOutput=False)
    out_ext = nc.declare_dram_parameter("out", [QTOT, D], BF, isOutput=True)

    with ExitStack() as ctx:
        tc = ctx.enter_context(tile.TileContext(nc))
        const = ctx.enter_context(tc.tile_pool(name="const", bufs=1))
        wpool = ctx.enter_context(tc.tile_pool(name="w", bufs=2))
        cxpool = ctx.enter_context(tc.tile_pool(name="cx", bufs=1))
        cqpool = ctx.enter_context(tc.tile_pool(name="cq", bufs=1))
        ktp = ctx.enter_context(tc.tile_pool(name="kt", bufs=1))
        vtp = ctx.enter_context(tc.tile_pool(name="vt", bufs=1))
        qtp = ctx.enter_context(tc.tile_pool(name="qt", bufs=1))
        pbp = ctx.enter_context(tc.tile_pool(name="pb", bufs=1))
        stgp = ctx.enter_context(tc.tile_pool(name="stg", bufs=4))
        mpool = ctx.enter_context(tc.tile_pool(name="m", bufs=3))
        rpool = ctx.enter_context(tc.tile_pool(name="r", bufs=2))
        opool = ctx.enter_context(tc.tile_pool(name="o", bufs=3))
        dram = ctx.enter_context(tc.tile_pool(name="dram", bufs=1, space="DRAM"))

        qpos_sb = const.tile([P, QTOT], F32)
        kpos_sb = const.tile([P, NT], F32)
        ones_sb = const.tile([P, 8], BF)
        nc.gpsimd.dma_start(out=ones_sb, in_=onesd[:, :])
        nc.gpsimd.dma_start(out=kpos_sb, in_=kpos[:, :])
        nc.gpsimd.dma_start(out=qpos_sb, in_=qpos[:, :])

        cx_sb = [cxpool.tile([P, NH2], BF, tag=f"cx{d}", name=f"cx{d}") for d in range(DN)]
        cx8_sb = [cxpool.tile([P, 2, NH2], F8, tag=f"cx8{d}", name=f"cx8{d}") for d in range(DS)]
        cq_sb = [cqpool.tile([P, QBLK], BF, tag=f"cq{d}", name=f"cq{d}") for d in range(DN)]
        cq8_sb = [cqpool.tile([P, 2, QBLK], F8, tag=f"cq8{d}", name=f"cq8{d}") for d in range(DS)]

        # ---- input DMA, need-ordered per queue (K ops, then V's, then Q's) --
        wk_sb = [wpool.tile([P, D], BF, tag=f"w{d}", name=f"wk{d}") for d in range(DN)]
        wk8_sb = [wpool.tile([P, 2, D], F8, tag=f"w8{d}", name=f"wk8{d}") for d in range(DS)]
        for d in range(DN):
            nc.sync.dma_start(out=cx_sb[d], in_=ctx_bf[d])
        for d in range(2):
            nc.scalar.dma_start(out=wk_sb[d], in_=w_bf[1][d])
        for d in range(DS):
            nc.scalar.dma_start(out=wk8_sb[d], in_=w_f8[1][d])
            nc.scalar.dma_start(out=cx8_sb[d], in_=ctx_f8[d])
        for d in range(2, DN):
            nc.scalar.dma_start(out=wk_sb[d], in_=w_bf[1][d])
        wv_sb = [wpool.tile([P, D], BF, tag=f"w{d}", name=f"wv{d}") for d in range(DN)]
        wv8_sb = [wpool.tile([P, 2, D], F8, tag=f"w8{d}", name=f"wv8{d}") for d in range(DS)]
        for d in range(DN):
            nc.sync.dma_start(out=wv_sb[d], in_=w_bf[2][d])
        for d in range(DS):
            nc.sync.dma_start(out=wv8_sb[d], in_=w_f8[2][d])
        for d in range(DN):
            nc.gpsimd.dma_start(out=cq_sb[d], in_=ctxq_bf[d])
        for d in range(DS):
            nc.gpsimd.dma_start(out=cq8_sb[d], in_=ctxq_f8[d])
        wq_sb = [wpool.tile([P, D], BF, tag=f"w{d}", name=f"wq{d}") for d in range(DN)]
        wq8_sb = [wpool.tile([P, 2, D], F8, tag=f"w8{d}", name=f"wq8{d}") for d in range(DS)]
        for d in range(DN):
            nc.gpsimd.dma_start(out=wq_sb[d], in_=w_bf[0][d])
        for d in range(DS):
            nc.gpsimd.dma_start(out=wq8_sb[d], in_=w_f8[0][d])

        kt_sb = [ktp.tile([P, N], BF, tag=f"k{e}", name=f"k{e}") for e in range(DN)]
        vt_sb = [vtp.tile([P, D], BF, tag=f"v{n}", name=f"v{n}") for n in range(NT)]
        qt_sb = [qtp.tile([P, QTOT], BF, tag=f"q{e}", name=f"q{e}") for e in range(DN)]
        pb_sb = [pbp.tile([P, QBLK], BF, tag=f"pb{k}", name=f"pb{k}") for k in range(NT // 2)]

        kstag_d = dram.tile([DN, P, NH], BF, name="kstag")
        kgath_d = dram.tile([2, DN, P, NH], BF, name="kgath")
        vstag_d = dram.tile([NHT, P, D], BF, name="vstag")
        vgath_d = dram.tile([2, NHT, P, D], BF, name="vgath")

        EH = DN // 2
        with tc.tile_pool(name="pp", bufs=8, space="PSUM") as pp:
            # K^T[e, n-own]: low 512 keys bf16, high 512 fp8 DoubleRow
            for eh in range(2):
                pss = {}
                for ei in range(EH):
                    for ci in range(2):
                        pss[ei, ci] = pp.tile([P, CH], F32, tag="pp", name="psk")
                for d in range(DN):
                    for ei in range(EH):
                        e = eh * EH + ei
                        nc.tensor.matmul(pss[ei, 0],
                                         lhsT=wk_sb[d][:, e * P:(e + 1) * P],
                                         rhs=cx_sb[d],
                                         start=(d == 0), stop=(d == DN - 1))
                for d in range(DS):
                    for ei in range(EH):
                        e = eh * EH + ei
                        nc.tensor.matmul(pss[ei, 1],
                                         lhsT=wk8_sb[d][:, :, e * P:(e + 1) * P],
                                         rhs=cx8_sb[d],
                                         start=(d == 0), stop=(d == DS - 1),
                                         perf_mode=DR)
                for ei in range(EH):
                    e = eh * EH + ei
                    stg = stgp.tile([P, NH], BF, tag="kstg", name="kstg")
                    nc.scalar.activation(stg[:, 0:CH], pss[ei, 0],
                                         AF.Identity, bias=0.0)
                    nc.scalar.activation(stg[:, CH:NH], pss[ei, 1],
                                         AF.Identity, bias=0.0, scale=1.0 / WS)
                    nc.scalar.dma_start(out=kstag_d[e], in_=stg)
            # exchange K halves within each pair as soon as staging is done
            # (rank order == global key order); the mesh runs behind V/Q-proj
            nc.gpsimd.collective_compute(
                "AllGather", mybir.AluOpType.bypass, replica_groups=GROUPS,
                ins=[kstag_d.opt()], outs=[kgath_d.opt()])
            # V[n-own, e]: first 4 own tiles bf16, last 4 fp8 DoubleRow
            for n_t in range(NHT):
                psv = [pp.tile([P, CH], F32, tag="pp", name="psv") for _ in range(2)]
                lo = n_t < NHT // 2
                if lo:
                    for d in range(DN):
                        for ec in range(2):
                            nc.tensor.matmul(psv[ec],
                                             lhsT=cx_sb[d][:, n_t * P:(n_t + 1) * P],
                                             rhs=wv_sb[d][:, ec * CH:(ec + 1) * CH],
                                             start=(d == 0), stop=(d == DN - 1))
                else:
                    nl = n_t - NHT // 2
                    for d in range(DS):
                        for ec in range(2):
                            nc.tensor.matmul(psv[ec],
                                             lhsT=cx8_sb[d][:, :, nl * P:(nl + 1) * P],
                                             rhs=wv8_sb[d][:, :, ec * CH:(ec + 1) * CH],
                                             start=(d == 0), stop=(d == DS - 1),
                                             perf_mode=DR)
                stg = stgp.tile([P, D], BF, tag="vstg", name="vstg")
                for ec in range(2):
                    nc.scalar.activation(stg[:, ec * CH:(ec + 1) * CH], psv[ec],
                                         AF.Identity, bias=0.0,
                                         scale=(1.0 if lo else 1.0 / WS))
                nc.sync.dma_start(out=vstag_d[n_t], in_=stg)
            # K readback (after V staging on the sync queue, waits on CC-K)
            for h in range(2):
                for e in range(DN):
                    nc.sync.dma_start(out=kt_sb[e][:, h * NH:(h + 1) * NH],
                                      in_=kgath_d[h][e])
            # exchange V halves (runs behind Q-proj and scores)
            nc.gpsimd.collective_compute(
                "AllGather", mybir.AluOpType.bypass, replica_groups=GROUPS,
                ins=[vstag_d.opt()], outs=[vgath_d.opt()])
            for h in range(2):
                for n_t in range(NHT):
                    nc.sync.dma_start(out=vt_sb[h * NHT + n_t], in_=vgath_d[h][n_t])
            # Q^T[e, q]: block A bf16, block B fp8 DoubleRow (overlaps CCs)
            for eh in range(2):
                pss = {}
                for ei in range(EH):
                    for qi in range(2):
                        pss[ei, qi] = pp.tile([P, CH], F32, tag="pp", name="psq")
                for d in range(DN):
                    for ei in range(EH):
                        e = eh * EH + ei
                        nc.tensor.matmul(pss[ei, 0],
                                         lhsT=wq_sb[d][:, e * P:(e + 1) * P],
                                         rhs=cq_sb[d],
                                         start=(d == 0), stop=(d == DN - 1))
                for d in range(DS):
                    for ei in range(EH):
                        e = eh * EH + ei
                        nc.tensor.matmul(pss[ei, 1],
                                         lhsT=wq8_sb[d][:, :, e * P:(e + 1) * P],
                                         rhs=cq8_sb[d],
                                         start=(d == 0), stop=(d == DS - 1),
                                         perf_mode=DR)
                for ei in range(EH):
                    e = eh * EH + ei
                    nc.scalar.activation(qt_sb[e][:, 0:CH], pss[ei, 0],
                                         AF.Identity, bias=0.0)
                    nc.scalar.activation(qt_sb[e][:, CH:QTOT], pss[ei, 1],
                                         AF.Identity, bias=0.0, scale=1.0 / WS)

        # ---------------- attention (bf16, everything SBUF-resident) --------
        # probs for k 0..7 (both blocks) recycle the cq buffers
        pa_sb = [cqpool.tile([P, QTOT], BF, tag=f"cq{k}", name=f"pa{k}")
                 for k in range(NT // 2)]
        with tc.tile_pool(name="ps_b", bufs=6, space="PSUM") as ps_b, \
             tc.tile_pool(name="ps_den", bufs=2, space="PSUM") as ps_den:
            # scores S^T[k, q] + exp + mask
            for k in range(NT):
                qcs = (0, 1) if k < NT // 2 else (1,)
                pss = {qc: ps_b.tile([P, CH], F32, tag="b", name="pss") for qc in qcs}
                for e in range(DN):
                    for qc in qcs:
                        nc.tensor.matmul(pss[qc], lhsT=kt_sb[e][:, k * P:(k + 1) * P],
                                         rhs=qt_sb[e][:, qc * CH:(qc + 1) * CH],
                                         start=(e == 0), stop=(e == DN - 1))
                for qc in qcs:
                    dst = (pa_sb[k][:, qc * CH:(qc + 1) * CH] if k < NT // 2
                           else pb_sb[k - NT // 2])
                    nc.scalar.activation(dst, pss[qc], AF.Exp, bias=0.0, scale=SCALE)
                    # block A masks low k-tiles; block B masks high k-tiles
                    if (k < NT // 2) == (qc == 0):
                        m = mpool.tile([P, CH], BF, tag="m", name="m")
                        nc.vector.tensor_scalar(m, qpos_sb[:, qc * CH:(qc + 1) * CH],
                                                kpos_sb[:, k:k + 1], None, OP.is_ge)
                        nc.vector.tensor_tensor(dst, dst, m, OP.mult)
            # PV + denominator + normalize per q-tile slot, causally trimmed;
            # block-A slots first (only need the earlier-arriving low V half)
            slots = ([(0, q_t) for q_t in reversed(range(QT))] +
                     [(1, q_t) for q_t in reversed(range(QT))])
            for si, (qb, q_t) in enumerate(slots):
                KT = capA[q_t] if qb == 0 else capB[q_t]
                pso = [ps_b.tile([P, CH], F32, tag="b", name="pso") for _ in range(2)]
                psd = ps_den.tile([P, 8], F32, tag="den", name="psd")
                for k in range(KT):
                    col = qb * CH + q_t * P
                    lhsT = (pa_sb[k][:, col:col + P] if k < NT // 2
                            else pb_sb[k - NT // 2][:, q_t * P:(q_t + 1) * P])
                    for ec in range(2):
                        nc.tensor.matmul(pso[ec], lhsT=lhsT,
                                         rhs=vt_sb[k][:, ec * CH:(ec + 1) * CH],
                                         start=(k == 0), stop=(k == KT - 1))
                    nc.tensor.matmul(psd, lhsT=lhsT, rhs=ones_sb,
                                     start=(k == 0), stop=(k == KT - 1))
                rec = rpool.tile([P, 1], F32, tag="rec", name="rec")
                nc.vector.reciprocal(rec, psd[:, 0:1])
                row = qb * QBLK + q_t * P
                ot = opool.tile([P, D], BF, tag="o", name="ot")
                for ec in range(2):
                    nc.vector.tensor_scalar_mul(ot[:, ec * CH:(ec + 1) * CH],
                                                pso[ec], rec)
                eng = nc.sync if si % 2 == 0 else nc.gpsimd
                eng.dma_start(out=out_ext[row:row + P, :], in_=ot)
    if fix_waits:
        _fix_matmul_waits(nc)
    return nc


def _bf_tiles(mat, np_bf):
    """[Dcontract, F] f32 -> [DN, ki=128, F] bf16 (d = dt*128 + ki)."""
    Dc, F = mat.shape
    return np.ascontiguousarray(mat.reshape(Dc // P, P, F).astype(np_bf))


def _fp8_interleave(mat, np_f8):
    """[Dcontract, F] f32 -> [DS, ki=128, ko=2, F] fp8 (d = ds*256+ko*128+ki,
    the packed layout fp8 DoubleRow matmuls contract over)."""
    Dc, F = mat.shape
    return np.ascontiguousarray(
        mat.reshape(Dc // 256, 2, P, F).transpose(0, 2, 1, 3).astype(np_f8))


def make_in_maps(context, W_qkv, b_qkv, n_cores=8):
    import ml_dtypes
    np_bf = ml_dtypes.bfloat16
    np_f8 = ml_dtypes.float8_e4m3
    context = np.asarray(context, np.float32)
    W_qkv = np.asarray(W_qkv, np.float32)
    b_qkv = np.asarray(b_qkv, np.float32)
    assert np.abs(b_qkv).max() == 0.0, "kernel folds zero qkv bias away"
    B, N, D = context.shape
    NT = N // P
    QBLK = N // 4
    QTOT = 2 * QBLK
    w8 = np.stack([_bf_tiles(W_qkv[:, p * D:(p + 1) * D], np_bf) for p in range(3)])
    wf8 = np.stack([_fp8_interleave(W_qkv[:, p * D:(p + 1) * D] * 32.0, np_f8)
                    for p in range(3)])
    kpos_a = (np.arange(NT)[None, :] * P + np.arange(P)[:, None]).astype(np.float32)
    kpos_a = np.ascontiguousarray(kpos_a)
    ones = np.ones((P, 8), np_bf)
    in_maps = []
    for c in range(n_cores):
        b, j = divmod(c, 2)
        sA = slice(j * QBLK, (j + 1) * QBLK)
        sB = slice((3 - j) * QBLK, (4 - j) * QBLK)
        ctx_b = context[b]
        # K/V are projected only for this core's key half (j=0: low, j=1: high);
        # within the half: first 512 keys bf16, last 512 fp8
        own = ctx_b[j * (N // 2):(j + 1) * (N // 2)]
        ctx8 = _bf_tiles(np.ascontiguousarray(own[:N // 4].T), np_bf)
        ctxf8 = _fp8_interleave(np.ascontiguousarray(own[N // 4:].T), np_f8)
        ctxq8 = _bf_tiles(np.ascontiguousarray(ctx_b[sA].T), np_bf)
        ctxqf8 = _fp8_interleave(np.ascontiguousarray(ctx_b[sB].T), np_f8)
        qpos_row = np.concatenate([np.arange(sA.start, sA.stop),
                                   np.arange(sB.start, sB.stop)])
        qpos_b = np.ascontiguousarray(
            np.broadcast_to(qpos_row.astype(np.float32), (P, QTOT)))
        in_maps.append({
            "ctx_bf": ctx8, "ctx_f8": ctxf8, "ctxq_bf": ctxq8,
            "ctxq_f8": ctxqf8, "w_bf": w8, "w_f8": wf8,
            "qpos": qpos_b, "kpos": kpos_a, "onesd": ones,
        })
    return in_maps


def assemble(results, B, N, D):
    QBLK = N // 4
    out = np.zeros((B, N, D), np.float32)
    for c, res in enumerate(results):
        b, j = divmod(c, 2)
        o = np.asarray(res["out"], dtype=np.float32)
        out[b, j * QBLK:(j + 1) * QBLK] = o[:QBLK]
        out[b, (3 - j) * QBLK:(4 - j) * QBLK] = o[QBLK:]
    return out


def run(inputs, trace=False, **spmd_kwargs):
    context = np.asarray(inputs["context"])
    B, N, D = context.shape
    nc = build(N, D)
    in_maps = make_in_maps(context, inputs["W_qkv"], inputs["b_qkv"], n_cores=8)
    res = run_bass_kernel_spmd(nc, in_maps, core_ids=list(range(8)), trace=trace, **spmd_kwargs)
    out = assemble(res.results, B, N, D)
    return out, res


def kernel(context, W_qkv, b_qkv):
    out, _ = run({"context": context, "W_qkv": W_qkv, "b_qkv": b_qkv})
    return out


# revision 16
# speedup vs baseline: 1.1905x; 1.1905x over previous
"""Causal self-attention (QKV projection + softmax(QK^T/sqrt(N)) @ V) on 8 TRN2
NeuronCores.

Sharding: core c = 2*b + j handles batch element b (of 4) and half the query
rows: block A = rows [j*512,(j+1)*512), block B = rows [(3-j)*512,(4-j)*512)
(mirrored blocks balance the causal triangle). Uniform SPMD schedule; per-core
causal masks (built on-device from shipped position vectors) zero invalid keys.

v4 scheme (bf16 everywhere + pairwise K/V projection dedup):
- Core j of each pair projects K^T/V only for its half of the keys (host ships
  that half of the context pre-transposed), then the halves are exchanged with
  a pairwise HBM AllGather; both cores read the gathered copies back in global
  key order, so all addressing stays SPMD-uniform.
- Phase order K -> V -> Q -> scores -> PV hides both collectives behind the
  Q projection and scores; PV runs block-A slots first (they only need the
  low half of V, which lands earlier).
- Scores are computed transposed S^T[k,q] = (K^T tile).T @ Q^T, softmax runs
  without max-subtraction, denominators come from a ones-vector matmul, P^T
  feeds PV directly, PV contraction is causally trimmed per q-tile slot.
"""

import math
from contextlib import ExitStack

import numpy as np

import concourse.bass as bass
import concourse.mybir as mybir
import concourse.tile as tile
from concourse.bass_utils import run_bass_kernel_spmd
from concourse.tile_rust import add_dep_helper

P = 128
CH = 512          # free-dim chunk (max fp32 moving operand / one PSUM bank)


def _fix_matmul_waits(nc):
    """Walrus codegen has a small per-instruction sync-wait slot budget (one
    for a self-loading matmul's LDWEIGHTS half, similar for ACT etc).  Move
    extra waits onto NoOps inserted just before the instruction on the same
    engine — per-engine program order (and thus semantics) is unchanged."""
    skip = (mybir.InstEventSemaphore, mybir.InstNoOp,
            mybir.InstUnconditionalBranch, mybir.InstCall)
    for func in nc.m.functions:
        for bb in func.blocks:
            il = bb.instructions
            new = []
            changed = False
            for inst in il:
                si = getattr(inst, "sync_info", None)
                if (si and si.on_wait and len(si.on_wait) > 1
                        and not isinstance(inst, skip)):
                    waits = list(si.on_wait)
                    for wi, w in enumerate(waits[:-1]):
                        nop = mybir.InstNoOp(
                            name=f"{inst.name}-wfix{wi}", engine=inst.engine,
                            sync_info=mybir.SyncInfo(on_wait=[w], on_update=[]),
                            text_hint="waitfix")
                        new.append(nop)
                    inst.sync_info = mybir.SyncInfo(
                        on_wait=[waits[-1]], on_update=list(si.on_update or []))
                    changed = True
                new.append(inst)
            if changed:
                bb.instructions = new


def build(N=2048, D=1024, fix_waits=True, **bass_kwargs):
    NT = N // P            # 16 key tiles
    NH = N // 2            # keys owned per core (1024)
    NHT = NH // P          # 8 owned key tiles
    DN = D // P            # 8 contraction / e-tiles
    QTOT = N // 2          # query rows per core (1024)
    QBLK = QTOT // 2       # rows per query block (512)
    QT = QBLK // P         # q-tiles per block (4)
    SCALE = 1.0 / math.sqrt(N)
    BF = mybir.dt.bfloat16
    F32 = mybir.dt.float32
    AF = mybir.ActivationFunctionType
    OP = mybir.AluOpType
    GROUPS = [[2 * b, 2 * b + 1] for b in range(4)]

    # causal PV contraction capacity per (block, q-tile) slot: max over the
    # j=0/j=1 occupant of that slot (uniform SPMD program, per-core data)
    capA = [QT + 1 + qt for qt in range(QT)]            # 5,6,7,8
    capB = [NT - 3 + qt for qt in range(QT)]            # 13,14,15,16

    nc = bass.Bass(num_devices=8, **bass_kwargs)

    ctx_bf = nc.declare_dram_parameter("ctx_bf", [DN, P, NH], BF, isOutput=False)
    ctxq_bf = nc.declare_dram_parameter("ctxq_bf", [DN, P, QTOT], BF, isOutput=False)
    w_bf = nc.declare_dram_parameter("w_bf", [3, DN, P, D], BF, isOutput=False)
    qpos = nc.declare_dram_parameter("qpos", [P, QTOT], F32, isOutput=False)
    kpos = nc.declare_dram_parameter("kpos", [P, NT], F32, isOutput=False)
    onesd = nc.declare_dram_parameter("onesd", [P, 8], BF, isOutput=False)
    out_ext = nc.declare_dram_parameter("out", [QTOT, D], BF, isOutput=True)

    with ExitStack() as ctx:
        tc = ctx.enter_context(tile.TileContext(nc))
        const = ctx.enter_context(tc.tile_pool(name="const", bufs=1))
        wpool = ctx.enter_context(tc.tile_pool(name="w", bufs=2))
        cxpool = ctx.enter_context(tc.tile_pool(name="cx", bufs=1))
        cqpool = ctx.enter_context(tc.tile_pool(name="cq", bufs=1))
        ktp = ctx.enter_context(tc.tile_pool(name="kt", bufs=1))
        vtp = ctx.enter_context(tc.tile_pool(name="vt", bufs=1))
        qtp = ctx.enter_context(tc.tile_pool(name="qt", bufs=1))
        pbp = ctx.enter_context(tc.tile_pool(name="pb", bufs=1))
        stgp = ctx.enter_context(tc.tile_pool(name="stg", bufs=4))
        mpool = ctx.enter_context(tc.tile_pool(name="m", bufs=3))
        rpool = ctx.enter_context(tc.tile_pool(name="r", bufs=2))
        opool = ctx.enter_context(tc.tile_pool(name="o", bufs=3))
        dram = ctx.enter_context(tc.tile_pool(name="dram", bufs=1, space="DRAM"))

        qpos_sb = const.tile([P, QTOT], F32)
        kpos_sb = const.tile([P, NT], F32)
        ones_sb = const.tile([P, 8], BF)
        nc.gpsimd.dma_start(out=ones_sb, in_=onesd[:, :])
        nc.gpsimd.dma_start(out=kpos_sb, in_=kpos[:, :])
        nc.gpsimd.dma_start(out=qpos_sb, in_=qpos[:, :])

        cx_sb = [cxpool.tile([P, NH], BF, tag=f"cx{d}", name=f"cx{d}") for d in range(DN)]
        cq_sb = [cqpool.tile([P, QTOT], BF, tag=f"cq{d}", name=f"cq{d}") for d in range(DN)]

        # ---- staged input DMA: K operands first, then V's, then Q's --------
        wk_sb = [wpool.tile([P, D], BF, tag=f"w{d}", name=f"wk{d}") for d in range(DN)]
        st0 = []
        for d in range(DN):
            st0.append(nc.scalar.dma_start(out=wk_sb[d], in_=w_bf[1][d]))
            st0.append(nc.sync.dma_start(out=cx_sb[d], in_=ctx_bf[d]))
        wv_sb = [wpool.tile([P, D], BF, tag=f"w{d}", name=f"wv{d}") for d in range(DN)]
        st1 = []
        for d in range(DN):
            bi = nc.scalar.dma_start(out=wv_sb[d], in_=w_bf[2][d])
            add_dep_helper(bi.ins, st0[-1].ins, sync=True, reason="dma stage1")
            st1.append(bi)
        for d in range(DN):
            bi = nc.gpsimd.dma_start(out=cq_sb[d], in_=ctxq_bf[d])
            add_dep_helper(bi.ins, st0[-1].ins, sync=True, reason="dma stage1")
            st1.append(bi)
        wq_sb = [wpool.tile([P, D], BF, tag=f"w{d}", name=f"wq{d}") for d in range(DN)]
        for d in range(DN):
            bi = nc.scalar.dma_start(out=wq_sb[d], in_=w_bf[0][d])
            add_dep_helper(bi.ins, st1[-1].ins, sync=True, reason="dma stage2")

        kt_sb = [ktp.tile([P, N], BF, tag=f"k{e}", name=f"k{e}") for e in range(DN)]
        vt_sb = [vtp.tile([P, D], BF, tag=f"v{n}", name=f"v{n}") for n in range(NT)]
        qt_sb = [qtp.tile([P, QTOT], BF, tag=f"q{e}", name=f"q{e}") for e in range(DN)]
        pb_sb = [pbp.tile([P, QBLK], BF, tag=f"pb{k}", name=f"pb{k}") for k in range(NT // 2)]

        kstag_d = dram.tile([DN, P, NH], BF, name="kstag")
        kgath_d = dram.tile([2, DN, P, NH], BF, name="kgath")
        vstag_d = dram.tile([NHT, P, D], BF, name="vstag")
        vgath_d = dram.tile([2, NHT, P, D], BF, name="vgath")

        EH = DN // 2
        with tc.tile_pool(name="pp", bufs=8, space="PSUM") as pp:
            # K^T[e, n-own]: W-stationary -> staging -> pairwise AllGather
            for eh in range(2):
                pss = {}
                for ei in range(EH):
                    for ci in range(2):
                        pss[ei, ci] = pp.tile([P, CH], F32, tag="pp", name="psk")
                for d in range(DN):
                    for ei in range(EH):
                        e = eh * EH + ei
                        for ci in range(2):
                            nc.tensor.matmul(pss[ei, ci],
                                             lhsT=wk_sb[d][:, e * P:(e + 1) * P],
                                             rhs=cx_sb[d][:, ci * CH:(ci + 1) * CH],
                                             start=(d == 0), stop=(d == DN - 1))
                for ei in range(EH):
                    e = eh * EH + ei
                    for ci in range(2):
                        stg = stgp.tile([P, CH], BF, tag="kstg", name="kstg")
                        nc.scalar.activation(stg, pss[ei, ci], AF.Identity, bias=0.0)
                        nc.sync.dma_start(out=kstag_d[e][:, ci * CH:(ci + 1) * CH],
                                          in_=stg)
            # exchange K halves within each pair as soon as staging is done
            # (rank order == global key order); the mesh runs behind V/Q-proj
            nc.gpsimd.collective_compute(
                "AllGather", mybir.AluOpType.bypass, replica_groups=GROUPS,
                ins=[kstag_d.opt()], outs=[kgath_d.opt()])
            # V[n-own, e]: ctx-stationary -> staging -> pairwise AllGather
            for n_t in range(NHT):
                psv = [pp.tile([P, CH], F32, tag="pp", name="psv") for _ in range(2)]
                for d in range(DN):
                    for ec in range(2):
                        nc.tensor.matmul(psv[ec], lhsT=cx_sb[d][:, n_t * P:(n_t + 1) * P],
                                         rhs=wv_sb[d][:, ec * CH:(ec + 1) * CH],
                                         start=(d == 0), stop=(d == DN - 1))
                for ec in range(2):
                    stg = stgp.tile([P, CH], BF, tag="vstg", name="vstg")
                    nc.scalar.activation(stg, psv[ec], AF.Identity, bias=0.0)
                    nc.sync.dma_start(out=vstag_d[n_t][:, ec * CH:(ec + 1) * CH],
                                      in_=stg)
            # K readback (after V staging on the sync queue, waits on CC-K)
            for h in range(2):
                for e in range(DN):
                    nc.sync.dma_start(out=kt_sb[e][:, h * NH:(h + 1) * NH],
                                      in_=kgath_d[h][e])
            # exchange V halves (runs behind Q-proj and scores)
            nc.gpsimd.collective_compute(
                "AllGather", mybir.AluOpType.bypass, replica_groups=GROUPS,
                ins=[vstag_d.opt()], outs=[vgath_d.opt()])
            for h in range(2):
                for n_t in range(NHT):
                    nc.sync.dma_start(out=vt_sb[h * NHT + n_t], in_=vgath_d[h][n_t])
            # Q^T[e, q]: W-stationary (overlaps the collectives)
            for eh in range(2):
                pss = {}
                for ei in range(EH):
                    for qi in range(2):
                        pss[ei, qi] = pp.tile([P, CH], F32, tag="pp", name="psq")
                for d in range(DN):
                    for ei in range(EH):
                        e = eh * EH + ei
                        for qi in range(2):
                            nc.tensor.matmul(pss[ei, qi],
                                             lhsT=wq_sb[d][:, e * P:(e + 1) * P],
                                             rhs=cq_sb[d][:, qi * CH:(qi + 1) * CH],
                                             start=(d == 0), stop=(d == DN - 1))
                for ei in range(EH):
                    e = eh * EH + ei
                    for qi in range(2):
                        nc.scalar.activation(qt_sb[e][:, qi * CH:(qi + 1) * CH],
                                             pss[ei, qi], AF.Identity, bias=0.0)

        # ---------------- attention (bf16, everything SBUF-resident) --------
        # probs for k 0..7 (both blocks) recycle the cq buffers
        pa_sb = [cqpool.tile([P, QTOT], BF, tag=f"cq{k}", name=f"pa{k}")
                 for k in range(NT // 2)]
        with tc.tile_pool(name="ps_b", bufs=6, space="PSUM") as ps_b, \
             tc.tile_pool(name="ps_den", bufs=2, space="PSUM") as ps_den:
            # scores S^T[k, q] + exp + mask
            for k in range(NT):
                qcs = (0, 1) if k < NT // 2 else (1,)
                pss = {qc: ps_b.tile([P, CH], F32, tag="b", name="pss") for qc in qcs}
                for e in range(DN):
                    for qc in qcs:
                        nc.tensor.matmul(pss[qc], lhsT=kt_sb[e][:, k * P:(k + 1) * P],
                                         rhs=qt_sb[e][:, qc * CH:(qc + 1) * CH],
                                         start=(e == 0), stop=(e == DN - 1))
                for qc in qcs:
                    dst = (pa_sb[k][:, qc * CH:(qc + 1) * CH] if k < NT // 2
                           else pb_sb[k - NT // 2])
                    nc.scalar.activation(dst, pss[qc], AF.Exp, bias=0.0, scale=SCALE)
                    # block A masks low k-tiles; block B masks high k-tiles
                    if (k < NT // 2) == (qc == 0):
                        m = mpool.tile([P, CH], BF, tag="m", name="m")
                        nc.vector.tensor_scalar(m, qpos_sb[:, qc * CH:(qc + 1) * CH],
                                                kpos_sb[:, k:k + 1], None, OP.is_ge)
                        nc.vector.tensor_tensor(dst, dst, m, OP.mult)
            # PV + denominator + normalize per q-tile slot, causally trimmed;
            # block-A slots first (only need the earlier-arriving low V half)
            slots = ([(0, q_t) for q_t in reversed(range(QT))] +
                     [(1, q_t) for q_t in reversed(range(QT))])
            for si, (qb, q_t) in enumerate(slots):
                KT = capA[q_t] if qb == 0 else capB[q_t]
                pso = [ps_b.tile([P, CH], F32, tag="b", name="pso") for _ in range(2)]
                psd = ps_den.tile([P, 8], F32, tag="den", name="psd")
                for k in range(KT):
                    col = qb * CH + q_t * P
                    lhsT = (pa_sb[k][:, col:col + P] if k < NT // 2
                            else pb_sb[k - NT // 2][:, q_t * P:(q_t + 1) * P])
                    for ec in range(2):
                        nc.tensor.matmul(pso[ec], lhsT=lhsT,
                                         rhs=vt_sb[k][:, ec * CH:(ec + 1) * CH],
                                         start=(k == 0), stop=(k == KT - 1))
                    nc.tensor.matmul(psd, lhsT=lhsT, rhs=ones_sb,
                                     start=(k == 0), stop=(k == KT - 1))
                rec = rpool.tile([P, 1], F32, tag="rec", name="rec")
                nc.vector.reciprocal(rec, psd[:, 0:1])
                row = qb * QBLK + q_t * P
                ot = opool.tile([P, D], BF, tag="o", name="ot")
                for ec in range(2):
                    nc.vector.tensor_scalar_mul(ot[:, ec * CH:(ec + 1) * CH],
                                                pso[ec], rec)
                eng = nc.sync if si % 2 == 0 else nc.gpsimd
                eng.dma_start(out=out_ext[row:row + P, :], in_=ot)
    if fix_waits:
        _fix_matmul_waits(nc)
    return nc


def _bf_tiles(mat, np_bf):
    """[Dcontract, F] f32 -> [DN, ki=128, F] bf16 (d = dt*128 + ki)."""
    Dc, F = mat.shape
    return np.ascontiguousarray(mat.reshape(Dc // P, P, F).astype(np_bf))


def make_in_maps(context, W_qkv, b_qkv, n_cores=8):
    import ml_dtypes
    np_bf = ml_dtypes.bfloat16
    context = np.asarray(context, np.float32)
    W_qkv = np.asarray(W_qkv, np.float32)
    b_qkv = np.asarray(b_qkv, np.float32)
    assert np.abs(b_qkv).max() == 0.0, "kernel folds zero qkv bias away"
    B, N, D = context.shape
    NT = N // P
    QBLK = N // 4
    QTOT = 2 * QBLK
    w8 = np.stack([_bf_tiles(W_qkv[:, p * D:(p + 1) * D], np_bf) for p in range(3)])
    kpos_a = (np.arange(NT)[None, :] * P + np.arange(P)[:, None]).astype(np.float32)
    kpos_a = np.ascontiguousarray(kpos_a)
    ones = np.ones((P, 8), np_bf)
    in_maps = []
    for c in range(n_cores):
        b, j = divmod(c, 2)
        sA = slice(j * QBLK, (j + 1) * QBLK)
        sB = slice((3 - j) * QBLK, (4 - j) * QBLK)
        ctx_b = context[b]
        # K/V are projected only for this core's key half (j=0: low, j=1: high)
        own = np.ascontiguousarray(ctx_b[j * (N // 2):(j + 1) * (N // 2)].T)
        ctx8 = _bf_tiles(own, np_bf)
        ctxq8 = _bf_tiles(
            np.ascontiguousarray(np.concatenate([ctx_b[sA], ctx_b[sB]], axis=0).T),
            np_bf)
        qpos_row = np.concatenate([np.arange(sA.start, sA.stop),
                                   np.arange(sB.start, sB.stop)])
        qpos_b = np.ascontiguousarray(
            np.broadcast_to(qpos_row.astype(np.float32), (P, QTOT)))
        in_maps.append({
            "ctx_bf": ctx8, "ctxq_bf": ctxq8, "w_bf": w8,
            "qpos": qpos_b, "kpos": kpos_a, "onesd": ones,
        })
    return in_maps


def assemble(results, B, N, D):
    QBLK = N // 4
    out = np.zeros((B, N, D), np.float32)
    for c, res in enumerate(results):
        b, j = divmod(c, 2)
        o = np.asarray(res["out"], dtype=np.float32)
        out[b, j * QBLK:(j + 1) * QBLK] = o[:QBLK]
        out[b, (3 - j) * QBLK:(4 - j) * QBLK] = o[QBLK:]
    return out


def run(inputs, trace=False, **spmd_kwargs):
    context = np.asarray(inputs["context"])
    B, N, D = context.shape
    nc = build(N, D)
    in_maps = make_in_maps(context, inputs["W_qkv"], inputs["b_qkv"], n_cores=8)
    res = run_bass_kernel_spmd(nc, in_maps, core_ids=list(range(8)), trace=trace, **spmd_kwargs)
    out = assemble(res.results, B, N, D)
    return out, res


def kernel(context, W_qkv, b_qkv):
    out, _ = run({"context": context, "W_qkv": W_qkv, "b_qkv": b_qkv})
    return out


# revision 17
# speedup vs baseline: 1.2555x; 1.0546x over previous
"""Causal self-attention (QKV projection + softmax(QK^T/sqrt(N)) @ V) on 8 TRN2
NeuronCores.

Sharding: core c = 2*b + j handles batch element b (of 4) and half the query
rows: block A = rows [j*512,(j+1)*512), block B = rows [(3-j)*512,(4-j)*512)
(mirrored blocks balance the causal triangle). Uniform SPMD schedule; per-core
causal masks (built on-device from shipped position vectors) zero invalid keys.

v4 scheme (bf16 everywhere + pairwise K/V projection dedup):
- Core j of each pair projects K^T/V only for its half of the keys (host ships
  that half of the context pre-transposed), then the halves are exchanged with
  a pairwise HBM AllGather; both cores read the gathered copies back in global
  key order, so all addressing stays SPMD-uniform.
- Phase order K -> V -> Q -> scores -> PV hides both collectives behind the
  Q projection and scores; PV runs block-A slots first (they only need the
  low half of V, which lands earlier).
- Scores are computed transposed S^T[k,q] = (K^T tile).T @ Q^T, softmax runs
  without max-subtraction, denominators come from a ones-vector matmul, P^T
  feeds PV directly, PV contraction is causally trimmed per q-tile slot.
"""

import math
from contextlib import ExitStack

import numpy as np

import concourse.bass as bass
import concourse.mybir as mybir
import concourse.tile as tile
from concourse.bass_utils import run_bass_kernel_spmd
from concourse.tile_rust import add_dep_helper

P = 128
CH = 512          # free-dim chunk (max fp32 moving operand / one PSUM bank)


def _fix_matmul_waits(nc):
    """Walrus codegen has a small per-instruction sync-wait slot budget (one
    for a self-loading matmul's LDWEIGHTS half, similar for ACT etc).  Move
    extra waits onto NoOps inserted just before the instruction on the same
    engine — per-engine program order (and thus semantics) is unchanged."""
    skip = (mybir.InstEventSemaphore, mybir.InstNoOp,
            mybir.InstUnconditionalBranch, mybir.InstCall)
    for func in nc.m.functions:
        for bb in func.blocks:
            il = bb.instructions
            new = []
            changed = False
            for inst in il:
                si = getattr(inst, "sync_info", None)
                if (si and si.on_wait and len(si.on_wait) > 1
                        and not isinstance(inst, skip)):
                    waits = list(si.on_wait)
                    for wi, w in enumerate(waits[:-1]):
                        nop = mybir.InstNoOp(
                            name=f"{inst.name}-wfix{wi}", engine=inst.engine,
                            sync_info=mybir.SyncInfo(on_wait=[w], on_update=[]),
                            text_hint="waitfix")
                        new.append(nop)
                    inst.sync_info = mybir.SyncInfo(
                        on_wait=[waits[-1]], on_update=list(si.on_update or []))
                    changed = True
                new.append(inst)
            if changed:
                bb.instructions = new


def build(N=2048, D=1024, fix_waits=True, **bass_kwargs):
    NT = N // P            # 16 key tiles
    NH = N // 2            # keys owned per core (1024)
    NHT = NH // P          # 8 owned key tiles
    DN = D // P            # 8 contraction / e-tiles
    QTOT = N // 2          # query rows per core (1024)
    QBLK = QTOT // 2       # rows per query block (512)
    QT = QBLK // P         # q-tiles per block (4)
    SCALE = 1.0 / math.sqrt(N)
    BF = mybir.dt.bfloat16
    F32 = mybir.dt.float32
    AF = mybir.ActivationFunctionType
    OP = mybir.AluOpType
    GROUPS = [[2 * b, 2 * b + 1] for b in range(4)]

    # causal PV contraction capacity per (block, q-tile) slot: max over the
    # j=0/j=1 occupant of that slot (uniform SPMD program, per-core data)
    capA = [QT + 1 + qt for qt in range(QT)]            # 5,6,7,8
    capB = [NT - 3 + qt for qt in range(QT)]            # 13,14,15,16

    nc = bass.Bass(num_devices=8, **bass_kwargs)

    ctx_bf = nc.declare_dram_parameter("ctx_bf", [DN, P, NH], BF, isOutput=False)
    ctxq_bf = nc.declare_dram_parameter("ctxq_bf", [DN, P, QTOT], BF, isOutput=False)
    w_bf = nc.declare_dram_parameter("w_bf", [3, DN, P, D], BF, isOutput=False)
    qpos = nc.declare_dram_parameter("qpos", [P, QTOT], F32, isOutput=False)
    kpos = nc.declare_dram_parameter("kpos", [P, NT], F32, isOutput=False)
    onesd = nc.declare_dram_parameter("onesd", [P, 8], BF, isOutput=False)
    out_ext = nc.declare_dram_parameter("out", [QTOT, D], BF, isOutput=True)

    with ExitStack() as ctx:
        tc = ctx.enter_context(tile.TileContext(nc))
        const = ctx.enter_context(tc.tile_pool(name="const", bufs=1))
        wpool = ctx.enter_context(tc.tile_pool(name="w", bufs=2))
        cxpool = ctx.enter_context(tc.tile_pool(name="cx", bufs=1))
        cqpool = ctx.enter_context(tc.tile_pool(name="cq", bufs=1))
        ktp = ctx.enter_context(tc.tile_pool(name="kt", bufs=1))
        vtp = ctx.enter_context(tc.tile_pool(name="vt", bufs=1))
        qtp = ctx.enter_context(tc.tile_pool(name="qt", bufs=1))
        pbp = ctx.enter_context(tc.tile_pool(name="pb", bufs=1))
        stgp = ctx.enter_context(tc.tile_pool(name="stg", bufs=4))
        mpool = ctx.enter_context(tc.tile_pool(name="m", bufs=3))
        rpool = ctx.enter_context(tc.tile_pool(name="r", bufs=2))
        opool = ctx.enter_context(tc.tile_pool(name="o", bufs=3))
        dram = ctx.enter_context(tc.tile_pool(name="dram", bufs=1, space="DRAM"))

        qpos_sb = const.tile([P, QTOT], F32)
        kpos_sb = const.tile([P, NT], F32)
        ones_sb = const.tile([P, 8], BF)
        nc.gpsimd.dma_start(out=ones_sb, in_=onesd[:, :])
        nc.gpsimd.dma_start(out=kpos_sb, in_=kpos[:, :])
        nc.gpsimd.dma_start(out=qpos_sb, in_=qpos[:, :])

        cx_sb = [cxpool.tile([P, NH], BF, tag=f"cx{d}", name=f"cx{d}") for d in range(DN)]
        cq_sb = [cqpool.tile([P, QTOT], BF, tag=f"cq{d}", name=f"cq{d}") for d in range(DN)]

        # ---- staged input DMA: K operands first, then V's, then Q's --------
        wk_sb = [wpool.tile([P, D], BF, tag=f"w{d}", name=f"wk{d}") for d in range(DN)]
        st0 = []
        for d in range(DN):
            st0.append(nc.scalar.dma_start(out=wk_sb[d], in_=w_bf[1][d]))
            st0.append(nc.sync.dma_start(out=cx_sb[d], in_=ctx_bf[d]))
        wv_sb = [wpool.tile([P, D], BF, tag=f"w{d}", name=f"wv{d}") for d in range(DN)]
        st1 = []
        for d in range(DN):
            bi = nc.gpsimd.dma_start(out=wv_sb[d], in_=w_bf[2][d])
            add_dep_helper(bi.ins, st0[-1].ins, sync=True, reason="dma stage1")
            st1.append(bi)
        for d in range(DN):
            bi = nc.gpsimd.dma_start(out=cq_sb[d], in_=ctxq_bf[d])
            add_dep_helper(bi.ins, st0[-1].ins, sync=True, reason="dma stage1")
            st1.append(bi)
        wq_sb = [wpool.tile([P, D], BF, tag=f"w{d}", name=f"wq{d}") for d in range(DN)]
        for d in range(DN):
            bi = nc.gpsimd.dma_start(out=wq_sb[d], in_=w_bf[0][d])
            add_dep_helper(bi.ins, st1[-1].ins, sync=True, reason="dma stage2")

        kt_sb = [ktp.tile([P, N], BF, tag=f"k{e}", name=f"k{e}") for e in range(DN)]
        vt_sb = [vtp.tile([P, D], BF, tag=f"v{n}", name=f"v{n}") for n in range(NT)]
        qt_sb = [qtp.tile([P, QTOT], BF, tag=f"q{e}", name=f"q{e}") for e in range(DN)]
        pb_sb = [pbp.tile([P, QBLK], BF, tag=f"pb{k}", name=f"pb{k}") for k in range(NT // 2)]

        kstag_d = dram.tile([DN, P, NH], BF, name="kstag")
        kgath_d = dram.tile([2, DN, P, NH], BF, name="kgath")
        vstag_d = dram.tile([NHT, P, D], BF, name="vstag")
        vgath_d = dram.tile([2, NHT, P, D], BF, name="vgath")

        EH = DN // 2
        with tc.tile_pool(name="pp", bufs=8, space="PSUM") as pp:
            # K^T[e, n-own]: W-stationary -> staging -> pairwise AllGather
            for eh in range(2):
                pss = {}
                for ei in range(EH):
                    for ci in range(2):
                        pss[ei, ci] = pp.tile([P, CH], F32, tag="pp", name="psk")
                for d in range(DN):
                    for ei in range(EH):
                        e = eh * EH + ei
                        for ci in range(2):
                            nc.tensor.matmul(pss[ei, ci],
                                             lhsT=wk_sb[d][:, e * P:(e + 1) * P],
                                             rhs=cx_sb[d][:, ci * CH:(ci + 1) * CH],
                                             start=(d == 0), stop=(d == DN - 1))
                for ei in range(EH):
                    e = eh * EH + ei
                    stg = stgp.tile([P, NH], BF, tag="kstg", name="kstg")
                    for ci in range(2):
                        nc.scalar.activation(stg[:, ci * CH:(ci + 1) * CH],
                                             pss[ei, ci], AF.Identity, bias=0.0)
                    nc.sync.dma_start(out=kstag_d[e], in_=stg)
            # exchange K halves within each pair as soon as staging is done
            # (rank order == global key order); the mesh runs behind V/Q-proj
            nc.gpsimd.collective_compute(
                "AllGather", mybir.AluOpType.bypass, replica_groups=GROUPS,
                ins=[kstag_d.opt()], outs=[kgath_d.opt()])
            # V[n-own, e]: ctx-stationary -> staging -> pairwise AllGather
            for n_t in range(NHT):
                psv = [pp.tile([P, CH], F32, tag="pp", name="psv") for _ in range(2)]
                for d in range(DN):
                    for ec in range(2):
                        nc.tensor.matmul(psv[ec], lhsT=cx_sb[d][:, n_t * P:(n_t + 1) * P],
                                         rhs=wv_sb[d][:, ec * CH:(ec + 1) * CH],
                                         start=(d == 0), stop=(d == DN - 1))
                stg = stgp.tile([P, D], BF, tag="vstg", name="vstg")
                for ec in range(2):
                    nc.scalar.activation(stg[:, ec * CH:(ec + 1) * CH],
                                         psv[ec], AF.Identity, bias=0.0)
                nc.sync.dma_start(out=vstag_d[n_t], in_=stg)
            # K readback (after V staging on the sync queue, waits on CC-K)
            for h in range(2):
                for e in range(DN):
                    nc.sync.dma_start(out=kt_sb[e][:, h * NH:(h + 1) * NH],
                                      in_=kgath_d[h][e])
            # exchange V halves (runs behind Q-proj and scores)
            nc.gpsimd.collective_compute(
                "AllGather", mybir.AluOpType.bypass, replica_groups=GROUPS,
                ins=[vstag_d.opt()], outs=[vgath_d.opt()])
            for h in range(2):
                for n_t in range(NHT):
                    nc.sync.dma_start(out=vt_sb[h * NHT + n_t], in_=vgath_d[h][n_t])
            # Q^T[e, q]: W-stationary (overlaps the collectives)
            for eh in range(2):
                pss = {}
                for ei in range(EH):
                    for qi in range(2):
                        pss[ei, qi] = pp.tile([P, CH], F32, tag="pp", name="psq")
                for d in range(DN):
                    for ei in range(EH):
                        e = eh * EH + ei
                        for qi in range(2):
                            nc.tensor.matmul(pss[ei, qi],
                                             lhsT=wq_sb[d][:, e * P:(e + 1) * P],
                                             rhs=cq_sb[d][:, qi * CH:(qi + 1) * CH],
                                             start=(d == 0), stop=(d == DN - 1))
                for ei in range(EH):
                    e = eh * EH + ei
                    for qi in range(2):
                        nc.scalar.activation(qt_sb[e][:, qi * CH:(qi + 1) * CH],
                                             pss[ei, qi], AF.Identity, bias=0.0)

        # ---------------- attention (bf16, everything SBUF-resident) --------
        # probs for k 0..7 (both blocks) recycle the cq buffers
        pa_sb = [cqpool.tile([P, QTOT], BF, tag=f"cq{k}", name=f"pa{k}")
                 for k in range(NT // 2)]
        with tc.tile_pool(name="ps_b", bufs=6, space="PSUM") as ps_b, \
             tc.tile_pool(name="ps_den", bufs=2, space="PSUM") as ps_den:
            # scores S^T[k, q] + exp + mask
            for k in range(NT):
                qcs = (0, 1) if k < NT // 2 else (1,)
                pss = {qc: ps_b.tile([P, CH], F32, tag="b", name="pss") for qc in qcs}
                for e in range(DN):
                    for qc in qcs:
                        nc.tensor.matmul(pss[qc], lhsT=kt_sb[e][:, k * P:(k + 1) * P],
                                         rhs=qt_sb[e][:, qc * CH:(qc + 1) * CH],
                                         start=(e == 0), stop=(e == DN - 1))
                for qc in qcs:
                    dst = (pa_sb[k][:, qc * CH:(qc + 1) * CH] if k < NT // 2
                           else pb_sb[k - NT // 2])
                    nc.scalar.activation(dst, pss[qc], AF.Exp, bias=0.0, scale=SCALE)
                    # block A masks low k-tiles; block B masks high k-tiles
                    if (k < NT // 2) == (qc == 0):
                        m = mpool.tile([P, CH], BF, tag="m", name="m")
                        nc.vector.tensor_scalar(m, qpos_sb[:, qc * CH:(qc + 1) * CH],
                                                kpos_sb[:, k:k + 1], None, OP.is_ge)
                        nc.vector.tensor_tensor(dst, dst, m, OP.mult)
            # PV + denominator + normalize per q-tile slot, causally trimmed;
            # block-A slots first (only need the earlier-arriving low V half)
            slots = ([(0, q_t) for q_t in reversed(range(QT))] +
                     [(1, q_t) for q_t in reversed(range(QT))])
            for si, (qb, q_t) in enumerate(slots):
                KT = capA[q_t] if qb == 0 else capB[q_t]
                pso = [ps_b.tile([P, CH], F32, tag="b", name="pso") for _ in range(2)]
                psd = ps_den.tile([P, 8], F32, tag="den", name="psd")
                for k in range(KT):
                    col = qb * CH + q_t * P
                    lhsT = (pa_sb[k][:, col:col + P] if k < NT // 2
                            else pb_sb[k - NT // 2][:, q_t * P:(q_t + 1) * P])
                    for ec in range(2):
                        nc.tensor.matmul(pso[ec], lhsT=lhsT,
                                         rhs=vt_sb[k][:, ec * CH:(ec + 1) * CH],
                                         start=(k == 0), stop=(k == KT - 1))
                    nc.tensor.matmul(psd, lhsT=lhsT, rhs=ones_sb,
                                     start=(k == 0), stop=(k == KT - 1))
                rec = rpool.tile([P, 1], F32, tag="rec", name="rec")
                nc.vector.reciprocal(rec, psd[:, 0:1])
                row = qb * QBLK + q_t * P
                ot = opool.tile([P, D], BF, tag="o", name="ot")
                for ec in range(2):
                    nc.vector.tensor_scalar_mul(ot[:, ec * CH:(ec + 1) * CH],
                                                pso[ec], rec)
                eng = nc.sync if si % 2 == 0 else nc.gpsimd
                eng.dma_start(out=out_ext[row:row + P, :], in_=ot)
    if fix_waits:
        _fix_matmul_waits(nc)
    return nc


def _bf_tiles(mat, np_bf):
    """[Dcontract, F] f32 -> [DN, ki=128, F] bf16 (d = dt*128 + ki)."""
    Dc, F = mat.shape
    return np.ascontiguousarray(mat.reshape(Dc // P, P, F).astype(np_bf))


def make_in_maps(context, W_qkv, b_qkv, n_cores=8):
    import ml_dtypes
    np_bf = ml_dtypes.bfloat16
    context = np.asarray(context, np.float32)
    W_qkv = np.asarray(W_qkv, np.float32)
    b_qkv = np.asarray(b_qkv, np.float32)
    assert np.abs(b_qkv).max() == 0.0, "kernel folds zero qkv bias away"
    B, N, D = context.shape
    NT = N // P
    QBLK = N // 4
    QTOT = 2 * QBLK
    w8 = np.stack([_bf_tiles(W_qkv[:, p * D:(p + 1) * D], np_bf) for p in range(3)])
    kpos_a = (np.arange(NT)[None, :] * P + np.arange(P)[:, None]).astype(np.float32)
    kpos_a = np.ascontiguousarray(kpos_a)
    ones = np.ones((P, 8), np_bf)
    in_maps = []
    for c in range(n_cores):
        b, j = divmod(c, 2)
        sA = slice(j * QBLK, (j + 1) * QBLK)
        sB = slice((3 - j) * QBLK, (4 - j) * QBLK)
        ctx_b = context[b]
        # K/V are projected only for this core's key half (j=0: low, j=1: high)
        own = np.ascontiguousarray(ctx_b[j * (N // 2):(j + 1) * (N // 2)].T)
        ctx8 = _bf_tiles(own, np_bf)
        ctxq8 = _bf_tiles(
            np.ascontiguousarray(np.concatenate([ctx_b[sA], ctx_b[sB]], axis=0).T),
            np_bf)
        qpos_row = np.concatenate([np.arange(sA.start, sA.stop),
                                   np.arange(sB.start, sB.stop)])
        qpos_b = np.ascontiguousarray(
            np.broadcast_to(qpos_row.astype(np.float32), (P, QTOT)))
        in_maps.append({
            "ctx_bf": ctx8, "ctxq_bf": ctxq8, "w_bf": w8,
            "qpos": qpos_b, "kpos": kpos_a, "onesd": ones,
        })
    return in_maps


def assemble(results, B, N, D):
    QBLK = N // 4
    out = np.zeros((B, N, D), np.float32)
    for c, res in enumerate(results):
        b, j = divmod(c, 2)
        o = np.asarray(res["out"], dtype=np.float32)
        out[b, j * QBLK:(j + 1) * QBLK] = o[:QBLK]
        out[b, (3 - j) * QBLK:(4 - j) * QBLK] = o[QBLK:]
    return out


def run(inputs, trace=False, **spmd_kwargs):
    context = np.asarray(inputs["context"])
    B, N, D = context.shape
    nc = build(N, D)
    in_maps = make_in_maps(context, inputs["W_qkv"], inputs["b_qkv"], n_cores=8)
    res = run_bass_kernel_spmd(nc, in_maps, core_ids=list(range(8)), trace=trace, **spmd_kwargs)
    out = assemble(res.results, B, N, D)
    return out, res


def kernel(context, W_qkv, b_qkv):
    out, _ = run({"context": context, "W_qkv": W_qkv, "b_qkv": b_qkv})
    return out
